# revision 18
# baseline (speedup 1.0000x reference)
"""ChannelCompressor (squeeze-excite over seq) Trainium2 Bass kernel.

reference semantics (per batch b):
    pooled = mean_s x[b, s, :]                 # [D]
    h      = gelu(pooled @ W1 + b1)            # [H]
    scale  = sigmoid(h @ W2 + b2)              # [D]
    out[b] = x[b] * scale[None, :]

Sharding: data-parallel over batch B=16 across 8 NeuronCores (2 batches
per core), excitor weights replicated. No collectives.

Per-core dataflow (memory-bound target; HBM floor = read 32 MiB +
write 32 MiB per core):
  - x[b] streamed into SBUF in 16 slabs of [128part, 2, 1024] (1 MiB
    HWDGE DMAs on the sync ring); the whole batch stays resident so x
    is read from HBM exactly once.
  - seq-mean via ones-stationary fp32 matmuls accumulated in a PSUM
    row [1, 1024] (TensorE, overlapped with the loads).
  - MLP in column layout: pooled row -> column via K=1 matmuls;
    per-partition biases fused into ScalarE activations.
  - scale column -> row via matmuls against identity; row -> [128,1024]
    broadcast via K=1 outer product with ones.
  - 32 in-place DVE tensor_mul, stores on the scalar HWDGE ring.

fp32 self-loading matmuls lower to an LDWEIGHTS+MATMUL pair whose LW
struct has a single sync-wait slot, so every matmul may carry at most
ONE semaphore wait.  Two measures keep each matmul to <=1 wait:
  - "fence" matmuls: a tiny matmul reading a freshly-produced operand
    right after its producer, so the PE engine clock observes the
    producer's semaphore before the real consumers run;
  - all small PSUM tiles are allocated ONCE and reused across batches
    via disjoint columns / restarted accumulation groups, so no
    tile-pool slot rotation injects extra cross-engine WAR waits.
"""

import sys

if "/opt/trn_rl_repo" not in sys.path:
    sys.path.insert(0, "/opt/trn_rl_repo")

from contextlib import ExitStack

import numpy as np

import concourse.bacc as bacc
import concourse.bass as bass
import concourse.tile as tile
from concourse import mybir

F32 = mybir.dt.float32
P = 128

B, S, D, H = 16, 4096, 1024, 128
N_CORES = 8
BPC = B // N_CORES  # batches per core


def build_nc(bpc=BPC, s=S, d=D, h=H, slab_t=2, x_bufs=20, act1=None, finalize=True):
    """Build the single-core Bass module for a [bpc, s, d] shard.

    Built on Bacc so finalize() runs move_matmul_waits_to_ldweights +
    generate_event_semaphores, which legalize any instruction that still
    carries more sync waits than its hardware struct allows.
    """
    if act1 is None:
        act1 = mybir.ActivationFunctionType.Gelu
    nchunk = d // P          # column chunks of the model dim
    nhalf = d // 512         # PSUM-bank halves of the model dim
    nslab = s // (slab_t * P)
    assert h == P and d % 512 == 0 and s % (slab_t * P) == 0

    nc = bacc.Bacc("TRN2")
    x = nc.dram_tensor("x", [bpc, s, d], F32, kind="ExternalInput")
    W1 = nc.dram_tensor("W1", [d, h], F32, kind="ExternalInput")
    b1 = nc.dram_tensor("b1", [h], F32, kind="ExternalInput")
    W2 = nc.dram_tensor("W2", [h, d], F32, kind="ExternalInput")
    b2 = nc.dram_tensor("b2", [d], F32, kind="ExternalInput")
    out = nc.dram_tensor("out", [bpc, s, d], F32, kind="ExternalOutput")

    with tile.TileContext(nc) as tc, ExitStack() as ctx:
        consts = ctx.enter_context(tc.tile_pool(name="consts", bufs=1))
        xpool = ctx.enter_context(tc.tile_pool(name="xpool", bufs=x_bufs))
        vec = ctx.enter_context(tc.tile_pool(name="vec", bufs=1))
        bcpool = ctx.enter_context(tc.tile_pool(name="bcpool", bufs=1))
        psum = ctx.enter_context(tc.tile_pool(name="psum", bufs=1, space="PSUM"))

        ones = consts.tile([P, P], F32)
        nc.vector.memset(ones, 1.0)
        ident = consts.tile([P, P], F32)
        nc.gpsimd.memset(ident, 0.0)
        nc.gpsimd.affine_select(
            out=ident,
            in_=ident,
            compare_op=mybir.AluOpType.not_equal,
            fill=1.0,
            base=0,
            pattern=[[-1, P]],
            channel_multiplier=1,
        )

        # W1 as lhsT chunks: W1_sb[k, j, m] = W1[j*128+k, m]
        W1_sb = consts.tile([P, nchunk, h], F32)
        nc.sync.dma_start(out=W1_sb, in_=W1[:, :].rearrange("(j k) m -> k j m", k=P))
        # W2 as lhsT chunks: W2_sb[m, j*128+k'] = W2[m, j*128+k']
        W2_sb = consts.tile([P, d], F32)
        nc.sync.dma_start(out=W2_sb, in_=W2[:, :])
        b1_sb = consts.tile([P, 1], F32)
        nc.sync.dma_start(out=b1_sb, in_=b1[:].rearrange("(k o) -> k o", o=1))
        # b2 in column layout: b2T_sb[k, j] = b2[j*128+k]
        b2T_sb = consts.tile([P, nchunk], F32)
        nc.sync.dma_start(out=b2T_sb, in_=b2[:].rearrange("(j k) -> k j", k=P))

        # Pre-observe the bias DMAs on their consumer engines so the real
        # consumers (gelu on ACT, b2-add on DVE) don't carry the DMA wait —
        # walrus' ACT/DVE structs also cap sync waits.
        b1_obs = consts.tile([P, 1], F32)
        nc.vector.tensor_copy(b1_obs, b1_sb)
        b2_obs = consts.tile([P, nchunk], F32)
        nc.vector.tensor_copy(b2_obs, b2T_sb)

        # PSUM tiles, each allocated ONCE (cross-batch reuse stays
        # intra-tensor => no slot-rotation WAR waits on matmuls).
        # Banks: colT 1 + pool 2 + srow 2 + bc 2 + fence 1 = 8 of 8.
        colT_ps = psum.tile([P, 32], F32, tag="colT")  # pooledT 0:8, h 8:9,
        #   scl 16:24
        pool_ps = psum.tile([1, d], F32, tag="pool")
        srow_ps = psum.tile([1, d], F32, tag="srow")
        bc_ps = psum.tile([P, d], F32, tag="bc")
        # fence outputs live in their own bank: colT is DVE-read (the b2 add),
        # and a PE write into a DVE-read bank gets a serialization wait
        fence_ps = psum.tile([1, 16], F32, tag="fence")

        fence_col = [0]

        def fence(lhsT, k):
            """Tiny matmul observing lhsT's producer on the PE clock."""
            c = fence_col[0]
            fence_col[0] += 1
            assert c < 16
            nc.tensor.matmul(
                fence_ps[0:1, c : c + 1],
                lhsT=lhsT,
                rhs=ones[0:k, 0:1],
                start=True,
                stop=True,
            )

        # observe the DVE (ones) and GpSimd (ident) sems before real work
        fence(ones[:, 0:1], P)
        fence(ident[:, 0:1], P)

        for b in range(bpc):
            xv = x[b].rearrange("(n t p) d -> n p t d", p=P, t=slab_t)
            ov = out[b].rearrange("(n t p) d -> n p t d", p=P, t=slab_t)

            # ---- load + pool (sum over seq) ----
            slabs = []
            for n in range(nslab):
                slab = xpool.tile([P, slab_t, d], F32, tag="slab")
                nc.sync.dma_start(out=slab, in_=xv[n])
                slabs.append(slab)
                if b > 0 and n == 0:
                    # absorb the DMA wait so the first pooling matmul of
                    # batch b only carries the PE WAW wait on pool_ps
                    fence(slab[:, 0, 0:1], P)
                for t in range(slab_t):
                    for hh in range(nhalf):
                        nc.tensor.matmul(
                            pool_ps[0:1, 512 * hh : 512 * (hh + 1)],
                            lhsT=ones[:, 0:1],
                            rhs=slab[:, t, 512 * hh : 512 * (hh + 1)],
                            start=(n == 0 and t == 0),
                            stop=(n == nslab - 1 and t == slab_t - 1),
                        )

            # pooled row (mean) to SBUF
            pooled_row = vec.tile([1, d], F32)
            nc.scalar.mul(pooled_row, pool_ps, 1.0 / s)
            fence(pooled_row[0:1, 0:1], 1)

            # row -> column layout: pooledT[k, j] = pooled[j*128+k]
            for j in range(nchunk):
                nc.tensor.matmul(
                    colT_ps[:, j : j + 1],
                    lhsT=pooled_row[0:1, P * j : P * (j + 1)],
                    rhs=ones[0:1, 0:1],
                    start=True,
                    stop=True,
                )
            pooledT_sb = vec.tile([P, nchunk], F32)
            nc.scalar.copy(pooledT_sb, colT_ps[:, 0:nchunk])
            fence(pooledT_sb[:, 0:1], P)

            # h = gelu(W1.T @ pooled + b1), column [128, 1]
            for j in range(nchunk):
                nc.tensor.matmul(
                    colT_ps[:, 8:9],
                    lhsT=W1_sb[:, j, :],
                    rhs=pooledT_sb[:, j : j + 1],
                    start=(j == 0),
                    stop=(j == nchunk - 1),
                )
            # bias-add on DVE (absorbs the PE wait; a bias'd ACT op has only
            # one free sync-wait slot), then a plain 1-wait gelu on ACT
            hpre_sb = vec.tile([P, 1], F32)
            nc.vector.tensor_scalar_add(hpre_sb, colT_ps[:, 8:9], b1_sb)
            hT_sb = vec.tile([P, 1], F32)
            nc.scalar.activation(hT_sb, hpre_sb, act1)
            fence(hT_sb, P)

            # scale_pre columns: sclT[k', j] = sum_m W2[m, j*128+k'] * h[m]
            for j in range(nchunk):
                nc.tensor.matmul(
                    colT_ps[:, 16 + j : 17 + j],
                    lhsT=W2_sb[:, P * j : P * (j + 1)],
                    rhs=hT_sb,
                    start=True,
                    stop=True,
                )
            sclT_sb = vec.tile([P, nchunk], F32, bufs=2)
            nc.vector.tensor_add(sclT_sb, colT_ps[:, 16 : 16 + nchunk], b2T_sb)
            sclS_sb = vec.tile([P, nchunk], F32)
            nc.scalar.activation(
                sclS_sb, sclT_sb, mybir.ActivationFunctionType.Sigmoid
            )
            fence(sclS_sb[:, 0:1], P)

            # column -> row: srow[0, j*128+n'] = sclS[n', j]
            for j in range(nchunk):
                nc.tensor.matmul(
                    srow_ps[0:1, P * j : P * (j + 1)],
                    lhsT=sclS_sb[:, j : j + 1],
                    rhs=ident[:, :],
                    start=True,
                    stop=True,
                )
            srow_sb = vec.tile([1, d], F32)
            nc.scalar.copy(srow_sb, srow_ps)
            fence(srow_sb[0:1, 0:1], 1)

            # broadcast row across partitions: bc[p, n] = srow[0, n]
            for hh in range(nhalf):
                nc.tensor.matmul(
                    bc_ps[:, 512 * hh : 512 * (hh + 1)],
                    lhsT=ones[0:1, :],
                    rhs=srow_sb[0:1, 512 * hh : 512 * (hh + 1)],
                    start=True,
                    stop=True,
                )
            scale_bc = bcpool.tile([P, d], F32)
            nc.scalar.copy(scale_bc, bc_ps)
            # observe scale_bc on the DVE clock so the muls only carry
            # their slab's DMA wait
            bc_obs = vec.tile([P, 1], F32)
            nc.vector.tensor_copy(bc_obs, scale_bc[:, 0:1])

            # ---- multiply + store ----
            for n, slab in enumerate(slabs):
                for t in range(slab_t):
                    nc.vector.tensor_mul(slab[:, t, :], slab[:, t, :], scale_bc)
                nc.scalar.dma_start(out=ov[n], in_=slab)

    if finalize:
        nc.finalize()
    return nc


def _shard_inputs(inputs):
    x = np.ascontiguousarray(np.asarray(inputs["x"], dtype=np.float32))
    W1 = np.ascontiguousarray(np.asarray(inputs["W1"], dtype=np.float32))
    b1 = np.ascontiguousarray(np.asarray(inputs["b1"], dtype=np.float32))
    W2 = np.ascontiguousarray(np.asarray(inputs["W2"], dtype=np.float32))
    b2 = np.ascontiguousarray(np.asarray(inputs["b2"], dtype=np.float32))
    in_maps = []
    for c in range(N_CORES):
        in_maps.append(
            {
                "x": np.ascontiguousarray(x[c * BPC : (c + 1) * BPC]),
                "W1": W1,
                "b1": b1,
                "W2": W2,
                "b2": b2,
            }
        )
    return in_maps


def _run(inputs, trace=False, **kwargs):
    from concourse.bass_utils import run_bass_kernel_spmd

    nc = build_nc()
    in_maps = _shard_inputs(inputs)
    res = run_bass_kernel_spmd(
        nc, in_maps, core_ids=list(range(N_CORES)), trace=trace, **kwargs
    )
    full = np.concatenate([r["out"] for r in res.results], axis=0)
    return full, res


def kernel(**inputs) -> np.ndarray:
    full, _ = _run(inputs, trace=False)
    return full
